# revision 14
# baseline (speedup 1.0000x reference)
"""GumbelSparseAttention Trainium2 kernel (8-core SPMD, head-sharded).

Key insight: the reference's straight-through gumbel-softmax mask is numerically
a hard one-hot, so softmax over the -inf-masked scores puts probability 1.0 on
exactly one key per (b, h, q). The q@k^T scores, k-projection and softmax are
dead code. The computation reduces to:
    q = query @ Wq.T               (only the 128 cols this core's 2 heads use)
    logits_h = q_h @ Wg.T
    idx = argmax(logits_h + gumbel_h)         (per (b, h, query-row))
    attn[:, h] = (value @ Wv.T)[idx, h-cols]  (row gather)
    out_partial = attn_cols @ Wo[:, cols].T   (summed across cores on host)

Sharding: core c owns heads {2c, 2c+1} = feature columns [128c, 128c+128).
All [B,H,S,S]-sized work (gumbel add + argmax) is perfectly sharded; the
small projections are column/row-sharded per the head split. Partial out-proj
sums + all (zero) biases are reduced/folded on the host.
"""

import numpy as np

import concourse.bass as bass
import concourse.bacc as bacc
import concourse.mybir as mybir
import bass_rust
from concourse.tile import TileContext
from concourse.masks import make_identity
from concourse.bass_utils import run_bass_kernel_spmd

B, S, E, H, HD = 2, 1024, 1024, 16, 64
NCORES = 8
HPC = H // NCORES          # 2 heads per core
FC = HPC * HD              # 128 feature cols per core
f32 = mybir.dt.float32
f32r = mybir.dt.float32r
u32 = mybir.dt.uint32

USE_F32R = True            # float32r: 1 cyc/row matmul when N>=256 (vs 4 for fp32)


def _mm_cast(ap):
    return ap


def _build():
    nc = bacc.Bacc()
    qT = nc.dram_tensor("qT", [B, E, S], f32r, kind="ExternalInput")
    vT = nc.dram_tensor("vT", [B, E, S], f32r, kind="ExternalInput")
    wqT = nc.dram_tensor("wqT", [E, FC], f32r, kind="ExternalInput")
    wvT = nc.dram_tensor("wvT", [E, FC], f32r, kind="ExternalInput")
    wgT = nc.dram_tensor("wgT", [HD, S], f32r, kind="ExternalInput")
    woT = nc.dram_tensor("woT", [FC, E], f32r, kind="ExternalInput")
    gum = nc.dram_tensor("gum", [B, HPC, S, S], f32, kind="ExternalInput")
    out = nc.dram_tensor("out", [B, S, E], f32, kind="ExternalOutput")
    vrows = nc.dram_tensor("vrows", [B * S, FC], f32)  # v-proj rows, gather table

    with TileContext(nc) as tc:
        with (
            tc.tile_pool(name="const", bufs=1) as const,
            tc.tile_pool(name="qin", bufs=3) as qin,
            tc.tile_pool(name="vin", bufs=3) as vin,
            tc.tile_pool(name="vmid", bufs=2) as vmid,
            tc.tile_pool(name="vrowt", bufs=3) as vrowt,
            tc.tile_pool(name="gumb", bufs=3) as gumb,
            tc.tile_pool(name="ltmp", bufs=3) as ltmp,
            tc.tile_pool(name="mx8", bufs=4) as mx8,
            tc.tile_pool(name="gat", bufs=3) as gat,
            tc.tile_pool(name="att", bufs=3) as att,
            tc.tile_pool(name="osb", bufs=3) as osb,
            tc.tile_pool(name="psA", bufs=2, space="PSUM") as psA,
            tc.tile_pool(name="psO", bufs=1, space="PSUM") as psO,
            tc.tile_pool(name="psB", bufs=2, space="PSUM") as psB,
        ):
            # ---- constants ----
            wq_sb = const.tile([128, E], f32r, tag="wq")
            wv_sb = const.tile([128, E], f32r, tag="wv")
            nc.sync.dma_start(wq_sb[:].rearrange("p (c f) -> p c f", f=FC),
                              wqT.rearrange("(c p) f -> p c f", p=128))
            nc.sync.dma_start(wv_sb[:].rearrange("p (c f) -> p c f", f=FC),
                              wvT.rearrange("(c p) f -> p c f", p=128))
            # Wg.T duplicated on both partition halves so each head's q slice
            # (base partition 0 / 64) has a same-base rhs.
            wg_sb = const.tile([128, S], f32r, tag="wg")
            nc.sync.dma_start(wg_sb[0:HD, :], wgT[:])
            nc.sync.dma_start(wg_sb[HD:128, :], wgT[:])
            wo_sb = const.tile([128, E], f32r, tag="wo")
            nc.sync.dma_start(wo_sb[:], woT[:])
            q_sb = const.tile([128, B * S], f32r, tag="qcols")      # q_colsT feature-major
            idx_all = const.tile([128, B * HPC * 8 * 8], u32, tag="idx")
            ident = const.tile([128, 128], f32, tag="ident")
            make_identity(nc, ident[:])

            # ---- q proj: q_colsT[f, b*S+s] = (query @ Wq[cols].T).T ----
            for b in range(B):
                ps0 = psA.tile([128, 512], f32, tag="big")
                ps1 = psA.tile([128, 512], f32, tag="big")
                pss = [ps0, ps1]
                for k in range(8):
                    rt_ = qin.tile([128, S], f32r, tag="qin")
                    nc.sync.dma_start(rt_[:], qT[b, k * 128:(k + 1) * 128, :])
                    for rs in range(2):
                        nc.tensor.matmul(pss[rs][:], lhsT=_mm_cast(wq_sb[:, k * 128:(k + 1) * 128]),
                                         rhs=_mm_cast(rt_[:, rs * 512:(rs + 1) * 512]),
                                         start=(k == 0), stop=(k == 7))
                for rs in range(2):
                    nc.scalar.copy(q_sb[:, (b * 2 + rs) * 512:(b * 2 + rs + 1) * 512], pss[rs][:])

            # ---- v proj (feat-major) + PE transpose -> row-major vrows in DRAM ----
            vw_insts = [[], []]
            for b in range(B):
                ps0 = psA.tile([128, 512], f32, tag="big")
                ps1 = psA.tile([128, 512], f32, tag="big")
                pss = [ps0, ps1]
                for k in range(8):
                    vt_ = vin.tile([128, S], f32r, tag="vin")
                    nc.sync.dma_start(vt_[:], vT[b, k * 128:(k + 1) * 128, :])
                    for rs in range(2):
                        nc.tensor.matmul(pss[rs][:], lhsT=_mm_cast(wv_sb[:, k * 128:(k + 1) * 128]),
                                         rhs=_mm_cast(vt_[:, rs * 512:(rs + 1) * 512]),
                                         start=(k == 0), stop=(k == 7))
                for rs in range(2):
                    vcT = vmid.tile([128, 512], f32, tag="vmid")
                    nc.scalar.copy(vcT[:], pss[rs][:])
                    for t in range(4):
                        tp = psB.tile([128, 128], f32, tag="small")
                        nc.tensor.transpose(tp[:], vcT[:, t * 128:(t + 1) * 128], ident[:])
                        vsb = vrowt.tile([128, 128], f32, tag="vrowt")
                        nc.vector.tensor_copy(vsb[:], tp[:])
                        r0 = b * S + rs * 512 + t * 128
                        wr = nc.sync.dma_start(vrows[r0:r0 + 128, :], vsb[:])
                        vw_insts[b].append(wr)

            # ---- fused per-row-tile: logits + gumbel argmax -> gather -> out proj ----
            for b in range(B):
                for rt in range(8):
                    # one DMA brings both heads' gumbel rows: [128, 2*1024]
                    gt = gumb.tile([128, HPC * S], f32, tag="gum")
                    nc.sync.dma_start(
                        gt[:].rearrange("p (h s) -> p h s", h=HPC),
                        gum[b, :, rt * 128:(rt + 1) * 128, :].rearrange("h p s -> p h s"))
                    for h in range(HPC):
                        lps = psA.tile([128, S], f32, tag="big")
                        lhs = q_sb[h * HD:(h + 1) * HD, b * S + rt * 128: b * S + (rt + 1) * 128]
                        wgh = wg_sb[h * HD:(h + 1) * HD, :]
                        nc.tensor.matmul(lps[:, 0:512], lhsT=_mm_cast(lhs),
                                         rhs=_mm_cast(wgh[:, 0:512]), start=True, stop=True)
                        nc.tensor.matmul(lps[:, 512:1024], lhsT=_mm_cast(lhs),
                                         rhs=_mm_cast(wgh[:, 512:1024]), start=True, stop=True)
                        tmp = ltmp.tile([128, S], f32, tag="ltmp")
                        nc.vector.tensor_add(tmp[:], lps[:], gt[:, h * S:(h + 1) * S])
                        m8 = mx8.tile([128, 8], f32, tag="m8")
                        nc.vector.max(out=m8[:], in_=tmp[:])
                        t = (b * HPC + h) * 8 + rt
                        nc.vector.max_index(out=idx_all[:, t * 8:(t + 1) * 8],
                                            in_max=m8[:], in_values=tmp[:])
                    # gather both heads' v rows for this row tile
                    gt_ = gat.tile([128, FC], f32, tag="gat")
                    t0 = (b * HPC + 0) * 8 + rt
                    t1 = (b * HPC + 1) * 8 + rt
                    g0 = nc.gpsimd.indirect_dma_start(
                        out=gt_[:, 0:HD], out_offset=None, in_=vrows[:],
                        in_offset=bass.IndirectOffsetOnAxis(ap=idx_all[:, t0 * 8:t0 * 8 + 1], axis=0),
                        element_offset=b * S * FC)
                    g1 = nc.gpsimd.indirect_dma_start(
                        out=gt_[:, HD:FC], out_offset=None, in_=vrows[:],
                        in_offset=bass.IndirectOffsetOnAxis(ap=idx_all[:, t1 * 8:t1 * 8 + 1], axis=0),
                        element_offset=b * S * FC + HD)
                    for wr in vw_insts[b]:
                        bass_rust.add_dep_helper(g0.ins, wr.ins, True, "vrows RAW")
                        bass_rust.add_dep_helper(g1.ins, wr.ins, True, "vrows RAW")
                    tp = psB.tile([128, 128], f32, tag="small")
                    nc.tensor.transpose(tp[:], gt_[:], ident[:])
                    at_ = att.tile([128, FC], f32r, tag="att")
                    nc.vector.tensor_copy(at_[:], tp[:])
                    ops = psO.tile([128, E], f32, tag="ops")
                    nc.tensor.matmul(ops[:, 0:512], lhsT=_mm_cast(at_[:]),
                                     rhs=_mm_cast(wo_sb[:, 0:512]), start=True, stop=True)
                    nc.tensor.matmul(ops[:, 512:1024], lhsT=_mm_cast(at_[:]),
                                     rhs=_mm_cast(wo_sb[:, 512:1024]), start=True, stop=True)
                    ob = osb.tile([128, E], f32, tag="osb")
                    nc.scalar.copy(ob[:], ops[:])
                    nc.sync.dma_start(out[b, rt * 128:(rt + 1) * 128, :], ob[:])
    nc.compile()
    return nc


_NC = None


def kernel(query, key, value, Wq, bq, Wk, bk, Wv, bv, Wg, bg, Wo, bo, gumbel_noise,
           _trace=False):
    global _NC
    if _NC is None:
        _NC = _build()
    nc = _NC

    qT = np.ascontiguousarray(np.asarray(query, np.float32).transpose(0, 2, 1))
    vT = np.ascontiguousarray(np.asarray(value, np.float32).transpose(0, 2, 1))
    Wq = np.asarray(Wq, np.float32); Wv = np.asarray(Wv, np.float32)
    Wg = np.asarray(Wg, np.float32); Wo = np.asarray(Wo, np.float32)
    bq = np.asarray(bq, np.float32); bg = np.asarray(bg, np.float32)
    bv = np.asarray(bv, np.float32); bo = np.asarray(bo, np.float32)
    gn = np.asarray(gumbel_noise, np.float32)
    wgT = np.ascontiguousarray(Wg.T)

    in_maps = []
    for c in range(NCORES):
        cols = slice(c * FC, (c + 1) * FC)
        gslice = np.ascontiguousarray(gn[:, c * HPC:(c + 1) * HPC])
        # fold bg and bq's contribution to logits into the gumbel tensor
        for i in range(HPC):
            hh = c * HPC + i
            row = bg + bq[hh * HD:(hh + 1) * HD] @ Wg.T
            if np.any(row):
                gslice[:, i] += row[None, None, :]
        in_maps.append({
            "qT": qT, "vT": vT,
            "wqT": np.ascontiguousarray(Wq[cols, :].T),
            "wvT": np.ascontiguousarray(Wv[cols, :].T),
            "wgT": wgT,
            "woT": np.ascontiguousarray(Wo[:, cols].T),
            "gum": gslice,
        })

    res = run_bass_kernel_spmd(nc, in_maps, core_ids=list(range(NCORES)), trace=_trace)
    kernel.last_results = res
    kernel.last_exec_ns = res.exec_time_ns

    out = np.zeros((B, S, E), np.float32)
    for r in res.results:
        out += r["out"]
    out += (bv @ Wo.T + bo)[None, None, :]
    return out.astype(np.float32)


kernel.last_results = None
kernel.last_exec_ns = None
